# revision 20
# baseline (speedup 1.0000x reference)
"""EntropyGuidedAttention Trainium2 Bass kernel.

Strategy (data-parallel over batch, 2 batches per core on 8 cores):

The q-projection (vf @ Wq.T, 2.4 GMAC/batch) is eliminated algebraically:
the attention logits only ever appear as q.k = vf^T (Wq^T (Wk text^T + bk))
+ bq^T(Wk text^T + bk), so we precompute
    kT2 = Wk @ textT + bk          [D, Q]   (tiny: Q=128)
    Y   = Wq^T @ kT2               [D, Q]
    c   = bq^T @ kT2               [1, Q]
per batch in the preamble, and the per-token logits become 6 accumulating
matmuls against the streaming vf tile (plus one K=1 matmul adding the c
row) - computed in the SAME single pass over vf that produces the entropy
partials.  vf is read from HBM exactly once; the [Q, N] logit matrix stays
in SBUF between the two passes.  Wq is used in natural layout (no
transpose); only Wk/Wv are transposed on the PE.

Entropy uses ent = log(Z) - T/Z with Z = sum(e^x), T = sum(x e^x), in full
fp32.  The Z/T partition-reductions accumulate over all 8 token groups into
a single [NG, G] PSUM tile each, via one-hot lhsT columns (group g's ones
column lands the sum in PSUM partition g) - no per-group copies or DRAM
round trips.  Token softmaxes skip max-subtraction: modulated logits are
O(1e-5) and feature logits are N(0,1), safe in fp32.

Precision: logit-path operands (Wq, Wk, bq, text-for-k) are fp8-e4m3 with a
x32 host-side pre-scale (the 1/1024 is folded into the te_eff
normalization); the ve*te modulation factor ~1e-6 crushes that rounding
error, as with the fp8 qT/kT of the previous version of this kernel.  The
value path (Wv, bv, text-for-v) is bf16; entropy and attention weights are
fp32.  Matmuls run float32r / bf16 / fp8 (all 1 cycle/row at free dim >=
256).

B=16, D=768, HxW=4096 tokens, Q=128.
"""

from contextlib import ExitStack

import ml_dtypes
import numpy as np

import concourse.bacc as bacc
import concourse.mybir as mybir
import concourse.tile as tile
from concourse.bass import ts
from concourse.bass_utils import run_bass_kernel_spmd
from concourse.masks import make_identity

F32 = mybir.dt.float32
F32R = mybir.dt.float32r
BF16 = mybir.dt.bfloat16
FP8 = mybir.dt.float8e4
AF = mybir.ActivationFunctionType
AX = mybir.AxisListType.X

N_CORES = 8
B, D, HH, WW, Q = 16, 768, 64, 64, 128
N = HH * WW                    # 4096 tokens per batch
BPC = B // N_CORES             # 2 batches per core
DC = D // 128                  # 6 feature chunks
G = 512                        # token group width
NG = N // G                    # 8 groups per batch
Q2 = Q * BPC                   # both batches' Q side by side
WSCALE = 32.0                  # host pre-scale on Wq/Wk/bq/bk (fp8 range)
SQRT_D = float(np.sqrt(np.float32(D)))
LOGIT_DIV = SQRT_D * WSCALE * WSCALE


def build_bass():
    nc = bacc.Bacc(None, target_bir_lowering=False)

    visual = nc.dram_tensor("visual", [BPC, D, N], F32R, kind="ExternalInput")
    text = nc.dram_tensor("text", [BPC, Q, D], F32, kind="ExternalInput")
    wq8 = nc.dram_tensor("wq8", [D, D], FP8, kind="ExternalInput")
    wkb = nc.dram_tensor("wkb", [D, D], BF16, kind="ExternalInput")
    wvb = nc.dram_tensor("wvb", [D, D], BF16, kind="ExternalInput")
    bq8 = nc.dram_tensor("bq8", [D], FP8, kind="ExternalInput")
    bk = nc.dram_tensor("bk", [D], F32, kind="ExternalInput")
    bvb = nc.dram_tensor("bvb", [D], BF16, kind="ExternalInput")
    out = nc.dram_tensor("out", [BPC, D, N], F32, kind="ExternalOutput")
    st_dram = nc.dram_tensor("st_scratch", [BPC, 1, 128], F32)
    c0_dram = nc.dram_tensor("c0_scratch", [BPC, 1, 1], F32)

    with tile.TileContext(nc) as tc, ExitStack() as ctx:
        K(ctx, tc, visual, text, wq8, wkb, wvb, bq8, bk, bvb, out,
          st_dram, c0_dram).emit()
    return nc


class K:
    def __init__(self, ctx, tc, visual, text, wq8, wkb, wvb, bq8, bk,
                 bvb, out, st_dram, c0_dram):
        self.ctx, self.tc, self.nc = ctx, tc, tc.nc
        self.visual, self.text = visual, text
        self.wq8, self.wkb, self.wvb = wq8, wkb, wvb
        self.bq8, self.bk, self.bvb = bq8, bk, bvb
        self.out = out
        self.st_dram, self.c0_dram = st_dram, c0_dram
        self.st = [dict() for _ in range(BPC)]   # per-batch tile state

    def emit(self):
        self.preamble()
        for g in range(NG):
            self.phase1_group(0, g)
        self.emit_T(0, NG - 1)
        self.finalize(0)
        # phase1(1) leads phase2(0) by 2 groups so finalize(1)'s DRAM
        # round-trips overlap the trailing phase2(0) groups.  p2_tail is
        # emitted FIRST each iteration so its za->zarow->zb->rzb chain isn't
        # queued behind the exps on ACT.
        for i in range(NG + 3):
            t = i - 3
            if 0 <= t < NG:
                self.p2_tail(0, t, evac_act=False)
            if i < NG:
                self.phase1_group(1, i)
            if i == NG:
                self.emit_T(1, NG - 1)
                self.finalize(1)
            h = i - 2
            if 0 <= h < NG:
                self.p2_head(0, h)
        # last block: no phase1 work left; ACT is idle so evacuate via
        # ean-on-Pool + plain ACT copies to unload DVE
        for g in range(NG + 1):
            if g > 0:
                self.p2_tail(1, g - 1, evac_act=True)
            if g < NG:
                self.p2_head(1, g)

    # ---------------- one-time preamble ----------------
    def preamble(self):
        nc, tc, ctx = self.nc, self.tc, self.ctx
        persist = ctx.enter_context(tc.tile_pool(name="persist", bufs=1))
        pb2 = ctx.enter_context(tc.tile_pool(name="pb2", bufs=2))
        self.persist, self.pb2 = persist, pb2

        ident = persist.tile([128, 128], F32, tag="ident")
        make_identity(nc, ident)
        ones_col_f = persist.tile([128, 1], F32, tag="ones_col_f")
        nc.vector.memset(ones_col_f, 1.0)
        ones_col = persist.tile([128, 1], F32R, tag="ones_col")
        nc.scalar.copy(out=ones_col, in_=ones_col_f)
        self.ones_col = ones_col
        ones_row_f = persist.tile([1, 128], F32, tag="ones_row_f")
        nc.vector.memset(ones_row_f, 1.0)
        ones_row = persist.tile([1, 128], F32R, tag="ones_row")
        nc.scalar.copy(out=ones_row, in_=ones_row_f)
        self.ones_row = ones_row
        ones_rowb = persist.tile([1, 128], BF16, tag="ones_rowb")
        nc.scalar.copy(out=ones_rowb, in_=ones_row_f)
        self.ones_rowb = ones_rowb
        ones_colb = persist.tile([128, 1], BF16, tag="ones_colb")
        nc.scalar.copy(out=ones_colb, in_=ones_col_f)
        self.ones_colb = ones_colb
        ones_g_f = persist.tile([1, G], F32, tag="ones_g_f")
        nc.vector.memset(ones_g_f, 1.0)
        ones_g = persist.tile([1, G], F32R, tag="ones_g")
        nc.scalar.copy(out=ones_g, in_=ones_g_f)
        self.ones_g = ones_g
        # one-hot columns: onehot[:, g, i] = 1.0 iff i == g.  Used as lhsT so
        # group g's partition-reduction lands in PSUM partition g.
        onehot_f = persist.tile([128, NG, NG], F32, tag="onehot_f")
        nc.vector.memset(onehot_f, 0.0)
        for g in range(NG):
            nc.vector.memset(onehot_f[:, g, g : g + 1], 1.0)
        onehot = persist.tile([128, NG, NG], F32R, tag="onehot")
        nc.scalar.copy(out=onehot, in_=onehot_f)
        self.onehot = onehot
        # rowhot[:, g, :]: [NG, 128] lhsT whose row g is ones -> broadcasts
        # row g of a [NG, *] rhs to all 128 output partitions.
        # out[p, (g, k)] = (p - g != 0) ? 0.0 : 1.0
        rowhot = persist.tile([NG, NG, 128], BF16, tag="rowhot")
        nc.gpsimd.memset(rowhot, 0.0)
        nc.gpsimd.affine_select(
            out=rowhot,
            in_=rowhot,
            compare_op=mybir.AluOpType.not_equal,
            fill=1.0,
            base=0,
            pattern=[[-1, NG], [0, 128]],
            channel_multiplier=1,
        )
        self.rowhot = rowhot

        # SBUF streaming pools created early so vf loads can pre-issue
        self.vf_pool = ctx.enter_context(tc.tile_pool(name="vf", bufs=3))
        self.es_pool = ctx.enter_context(tc.tile_pool(name="escr", bufs=3))
        self.xe_pool = ctx.enter_context(tc.tile_pool(name="xescr", bufs=13))
        self.at_pool = ctx.enter_context(tc.tile_pool(name="attn", bufs=2))
        self.oc_pool = ctx.enter_context(tc.tile_pool(name="outc", bufs=2))
        self.sm_pool = ctx.enter_context(tc.tile_pool(name="small", bufs=1))

        self.textTb = persist.tile([128, DC, Q2], BF16, tag="textTb")
        self.kT2 = persist.tile([128, DC, Q2], FP8, tag="kT2")
        self.y_t = persist.tile([128, DC, Q2], F32R, tag="y_t")
        self.c_row = persist.tile([1, Q2], F32R, tag="c_row")

        with tc.tile_pool(name="pre_sb", bufs=1) as pre_sb, \
             tc.tile_pool(name="pre_sbt", bufs=2) as pre_sbt, \
             tc.tile_pool(name="pre_ps", bufs=2, space="PSUM") as pre_ps, \
             tc.tile_pool(name="pre_ps2", bufs=2, space="PSUM") as pre_ps2:
            # text loads + transposes + text entropy (first: cheap DMA, long
            # dependent chain)
            for b in range(BPC):
                text_nat = pre_sbt.tile([Q, D], F32, tag="text_nat",
                                        name=f"text_nat{b}")
                nc.sync.dma_start(out=text_nat, in_=self.text.ap()[b])
                qb = ts(b, Q)
                for dc in range(DC):
                    pt = pre_ps.tile([128, G], F32, tag="pt")
                    nc.tensor.transpose(pt[:, :Q], text_nat[:, ts(dc, 128)],
                                        ident)
                    nc.scalar.copy(out=self.textTb[:, dc, qb], in_=pt[:, :Q])
                self.text_entropy(b, text_nat)

            # Wk load + transpose (bf16)
            identb = persist.tile([128, 128], BF16, tag="identb")
            nc.scalar.copy(out=identb, in_=ident)
            wkT = pre_sb.tile([128, DC, D], BF16, tag="wkT")
            self.wvT = persist.tile([128, DC, D], BF16, tag="wvT")
            wk_nat = pre_sb.tile([128, DC, D], BF16, tag="wk_nat")
            nc.sync.dma_start(
                out=wk_nat,
                in_=self.wkb.ap().rearrange("(c p) k -> p c k", p=128))

            # Wq natural (fp8; lhsT for Y contracts over its rows) + biases
            self.wq_nat = persist.tile([128, DC, D], FP8, tag="wq_nat")
            nc.sync.dma_start(
                out=self.wq_nat,
                in_=self.wq8.ap().rearrange("(c p) k -> p c k", p=128))
            self.bk_col = persist.tile([128, DC], F32, tag="bk_col")
            nc.sync.dma_start(
                out=self.bk_col,
                in_=self.bk.ap().rearrange("(c p) -> p c", p=128))
            self.bq_col = persist.tile([128, DC], FP8, tag="bq_col")
            nc.sync.dma_start(
                out=self.bq_col,
                in_=self.bq8.ap().rearrange("(c p) -> p c", p=128))

            # pre-issue the first two vf group loads (keeps DMA busy during
            # the rest of the preamble; phase1_group(0, 0/1) consumes these)
            self.vf_pre = []
            for g in range(2):
                vft = self.vf_pool.tile([128, DC, G], F32R, tag="vf")
                nc.sync.dma_start(
                    out=vft,
                    in_=self.visual.ap()[0].rearrange(
                        "(c p) n -> p c n", p=128)[:, :, ts(g, G)])
                self.vf_pre.append(vft)

            # Wv load + bv (value path; only needed by p2_tail, so last)
            wv_nat = pre_sb.tile([128, DC, D], BF16, tag="wv_nat")
            nc.sync.dma_start(
                out=wv_nat,
                in_=self.wvb.ap().rearrange("(c p) k -> p c k", p=128))
            self.bv_row = persist.tile([1, D], BF16, tag="bv_row")
            nc.sync.dma_start(out=self.bv_row,
                              in_=self.bvb.ap().rearrange("(a k) -> a k", a=1))

            for w_nat, wT in ((wk_nat, wkT), (wv_nat, self.wvT)):
                for kc in range(DC):
                    for jh in range(2):   # 3 transposes -> half-row of wT
                        pt = pre_ps.tile([128, G], BF16, tag="ptb")
                        for jx in range(3):
                            jc = jh * 3 + jx
                            nc.tensor.transpose(
                                pt[:, ts(jx, 128)],
                                w_nat[:, jc, ts(kc, 128)], identb)
                        nc.vector.tensor_scalar_add(
                            out=wT[:, kc, jh * 384 : (jh + 1) * 384],
                            in0=pt[:, :384], scalar1=0.0)

            # kT2 = WSCALE*(Wk @ textT + bk)   [dout, Q2]  (fp8)
            for jc in range(DC):
                kp = pre_ps2.tile([128, Q2], F32, tag="p2")
                for dc in range(DC):
                    nc.tensor.matmul(
                        kp, wkT[:, dc, ts(jc, 128)], self.textTb[:, dc, :],
                        start=(dc == 0), stop=(dc == DC - 1))
                nc.scalar.activation(
                    out=self.kT2[:, jc, :], in_=kp, func=AF.Identity,
                    scale=WSCALE, bias=self.bk_col[:, jc : jc + 1])

            # Y = Wq^T @ kT2  [din, Q2]  (f32r; Wq natural as lhsT)
            for ic in range(DC):
                yp = pre_ps2.tile([128, Q2], F32, tag="p2")
                for oc in range(DC):
                    nc.tensor.matmul(
                        yp, self.wq_nat[:, oc, ts(ic, 128)],
                        self.kT2[:, oc, :],
                        start=(oc == 0), stop=(oc == DC - 1))
                nc.scalar.copy(out=self.y_t[:, ic, :], in_=yp)

            # c = bq^T @ kT2  [1, Q2]
            cp = pre_ps2.tile([128, Q2], F32, tag="p2")
            for oc in range(DC):
                nc.tensor.matmul(cp[:1, :], self.bq_col[:, oc : oc + 1],
                                 self.kT2[:, oc, :],
                                 start=(oc == 0), stop=(oc == DC - 1))
            nc.scalar.copy(out=self.c_row, in_=cp[:1, :])

            # v = text @ Wv^T + bv  [Q, D] per batch  (bf16 path)
            for b in range(BPC):
                v_sb = pb2.tile([Q, D], BF16, tag="v_sb", name=f"v{b}")
                for jg, jw in ((0, G), (1, D - G)):
                    vp = pre_ps.tile([128, G], F32, tag="pt")
                    for dc in range(DC):
                        nc.tensor.matmul(
                            vp[:, :jw], self.textTb[:, dc, ts(b, Q)],
                            self.wvT[:, dc, jg * G : jg * G + jw],
                            start=(dc == 0), stop=False)
                    nc.tensor.matmul(
                        vp[:, :jw], self.ones_rowb,
                        self.bv_row[:, jg * G : jg * G + jw],
                        start=False, stop=True)
                    nc.scalar.copy(out=v_sb[:, jg * G : jg * G + jw],
                                   in_=vp[:, :jw])
                self.st[b]["v_sb"] = v_sb

        # streaming PSUM pools (opened after preamble PSUM pools close)
        self.mm_ps = ctx.enter_context(
            tc.tile_pool(name="mm_ps", bufs=4, space="PSUM"))
        self.lg_ps = ctx.enter_context(
            tc.tile_pool(name="lg_ps", bufs=2, space="PSUM"))
        self.zt_ps = ctx.enter_context(
            tc.tile_pool(name="zt_ps", bufs=1, space="PSUM"))

        for b in range(BPC):
            self.st[b]["lp"] = pb2.tile([Q, N], BF16, tag="lp", name=f"lp{b}")

    # ---------------- per-batch text entropy -> evt, St ----------------
    def text_entropy(self, b, text_f):
        nc = self.nc
        st = self.st[b]
        sm = self.sm_pool
        maxm = sm.tile([Q, 1], F32, tag="maxm", name=f"maxm{b}")
        nc.vector.reduce_max(out=maxm, in_=text_f, axis=AX)
        negm = sm.tile([Q, 1], F32, tag="negm", name=f"negm{b}")
        nc.vector.tensor_scalar_mul(out=negm, in0=maxm, scalar1=-1.0)
        et = sm.tile([Q, D], F32, tag="et", name=f"et{b}")
        zt = sm.tile([Q, 1], F32, tag="zt", name=f"zt{b}")
        nc.scalar.activation(out=et, in_=text_f, func=AF.Exp, bias=negm,
                             accum_out=zt)
        tt = sm.tile([Q, 1], F32, tag="tt", name=f"tt{b}")
        nc.vector.tensor_mul(out=et, in0=et, in1=text_f)
        nc.vector.reduce_sum(out=tt, in_=et, axis=AX)
        rzt = sm.tile([Q, 1], F32, tag="rzt", name=f"rzt{b}")
        nc.vector.reciprocal(out=rzt, in_=zt)
        t2 = sm.tile([Q, 1], F32, tag="t2", name=f"t2{b}")
        nc.vector.tensor_mul(out=t2, in0=tt, in1=rzt)
        lnz = sm.tile([Q, 1], F32, tag="lnz", name=f"lnz{b}")
        nc.scalar.activation(out=lnz, in_=zt, func=AF.Ln)
        ent_t = sm.tile([Q, 1], F32, tag="ent_t", name=f"ent_t{b}")
        nc.vector.tensor_sub(out=ent_t, in0=lnz, in1=t2)
        nc.vector.tensor_add(out=ent_t, in0=ent_t, in1=maxm)
        evt = self.pb2.tile([Q, 1], F32, tag="evt", name=f"evt{b}")
        nc.scalar.activation(out=evt, in_=ent_t, func=AF.Exp)
        st["evt"] = evt
        # S_t via DRAM round-trip (column -> row)
        nc.sync.dma_start(
            out=self.st_dram.ap()[b].rearrange("one p -> p one"), in_=evt)
        st_row = sm.tile([1, Q], F32, tag="st_row", name=f"strow{b}")
        nc.sync.dma_start(out=st_row, in_=self.st_dram.ap()[b])
        st_sb = self.pb2.tile([1, 1], F32, tag="st_sb", name=f"stsb{b}")
        nc.vector.reduce_sum(out=st_sb, in_=st_row, axis=AX)
        st["st_sb"] = st_sb

    # ------- phase 1 (per group): entropy partials + logits, one vf pass ----
    def phase1_group(self, b, g):
        nc = self.nc
        st = self.st[b]
        gs = slice(g * G, (g + 1) * G)
        if g == 0:
            st["zacc"] = self.zt_ps.tile([NG, G], F32, tag="zacc",
                                         name=f"zacc{b}")
            st["tacc"] = self.zt_ps.tile([NG, G], F32, tag="tacc",
                                         name=f"tacc{b}")
        if b == 0 and g < len(self.vf_pre):
            vf = self.vf_pre[g]
        else:
            vf = self.vf_pool.tile([128, DC, G], F32R, tag="vf")
            nc.sync.dma_start(
                out=vf,
                in_=self.visual.ap()[b].rearrange(
                    "(c p) n -> p c n", p=128)[:, :, gs],
            )
        vf_f = vf.bitcast(F32)

        # logits first on PE: they depend only on vf (+ the lp bank)
        lp_full = self.mm_ps.tile([128, G], F32, tag="mm")
        lp_ps = lp_full[:Q, :]
        for dc in range(DC):
            nc.tensor.matmul(lp_ps, self.y_t[:, dc, ts(b, Q)], vf[:, dc, :],
                             start=(dc == 0), stop=False)
        nc.tensor.matmul(lp_ps, self.c_row[:, ts(b, Q)], self.ones_g,
                         start=False, stop=True)

        # T matmuls for the PREVIOUS group: their xe inputs are long ready,
        # so the PE never stalls on the exp->xe chain of the current group
        if g > 0:
            self.emit_T(b, g - 1)

        oh = self.onehot[:, g, :]
        # xe engine split: batch 0 runs alone (first block) -> even DVE/Pool
        # split; batch 1 overlaps phase2(0) whose evacs load DVE -> mostly Pool
        xe_dve = (0, 2, 4) if b == 0 else (1,)
        st[f"xes{g}"] = []
        for dc in range(DC):
            ex = self.es_pool.tile([128, G], F32R, tag="ex")
            nc.scalar.activation(out=ex, in_=vf_f[:, dc, :], func=AF.Exp)
            xe = self.xe_pool.tile([128, G], F32R, tag="xe")
            eng = nc.vector if dc in xe_dve else nc.gpsimd
            eng.tensor_mul(out=xe, in0=ex.bitcast(F32), in1=vf_f[:, dc, :])
            st[f"xes{g}"].append(xe)
            nc.tensor.matmul(st["zacc"], oh, ex,
                             start=(g == 0 and dc == 0),
                             stop=(g == NG - 1 and dc == DC - 1))
        nc.vector.tensor_scalar_add(out=st["lp"][:, gs], in0=lp_ps,
                                    scalar1=0.0)

    def emit_T(self, b, g):
        nc = self.nc
        st = self.st[b]
        oh = self.onehot[:, g, :]
        for dc, xe in enumerate(st.pop(f"xes{g}")):
            nc.tensor.matmul(st["tacc"], oh, xe,
                             start=(g == 0 and dc == 0),
                             stop=(g == NG - 1 and dc == DC - 1))

    # ---------------- per-batch entropy finalize ----------------
    def finalize(self, b):
        nc = self.nc
        st = self.st[b]
        sm = self.sm_pool
        rz = sm.tile([NG, G], F32, tag="rz")
        nc.vector.reciprocal(out=rz, in_=st["zacc"])
        nc.vector.tensor_mul(out=rz, in0=st["tacc"], in1=rz)
        lnz = sm.tile([NG, G], F32, tag="lnz8")
        nc.scalar.activation(out=lnz, in_=st["zacc"], func=AF.Ln)
        nc.vector.tensor_sub(out=lnz, in0=lnz, in1=rz)
        exp_ent = self.pb2.tile([NG, G], BF16, tag="exp_ent", name=f"ee{b}")
        nc.scalar.activation(out=exp_ent, in_=lnz, func=AF.Exp)
        st["exp_ent"] = exp_ent

        sve_p = self.lg_ps.tile([128, G], F32, tag="lg")
        nc.tensor.matmul(sve_p[:1, :], self.ones_colb[:NG], exp_ent,
                         start=True, stop=True)
        sve_sb = sm.tile([1, 1], F32, tag="sve_sb")
        nc.vector.reduce_sum(out=sve_sb, in_=sve_p[:1, :], axis=AX)

        c0 = sm.tile([1, 1], F32, tag="c0")
        nc.vector.tensor_mul(out=c0, in0=st["st_sb"], in1=sve_sb)
        nc.vector.reciprocal(out=c0, in_=c0)
        nc.vector.tensor_scalar_mul(out=c0, in0=c0, scalar1=1.0 / LOGIT_DIV)
        nc.sync.dma_start(out=self.c0_dram.ap()[b], in_=c0)
        c0b = sm.tile([128, 1], F32, tag="c0b")
        nc.sync.dma_start(out=c0b,
                          in_=self.c0_dram.ap()[b].broadcast_to((128, 1)))
        te_eff = self.pb2.tile([Q, 1], F32, tag="te_eff", name=f"te{b}")
        nc.vector.tensor_mul(out=te_eff, in0=st["evt"], in1=c0b)
        st["te_eff"] = te_eff

    # ------- phase 2 (per group), split for software pipelining ----------
    def p2_head(self, b, g):
        nc = self.nc
        st = self.st[b]
        gs = slice(g * G, (g + 1) * G)
        veb = self.lg_ps.tile([128, G], F32, tag="lg")
        nc.tensor.matmul(veb, self.rowhot[:, g, :], st["exp_ent"],
                         start=True, stop=True)
        smod = self.at_pool.tile([Q, G], F32, tag="smod")
        nc.vector.tensor_mul(out=smod, in0=st["lp"][:, gs], in1=veb)
        ea = self.at_pool.tile([Q, G], BF16, tag="ea")
        nc.scalar.activation(out=ea, in_=smod, func=AF.Exp,
                             scale=st["te_eff"])
        st[f"ea{g}"] = ea

    def p2_tail(self, b, g, evac_act):
        nc = self.nc
        st = self.st[b]
        gs = slice(g * G, (g + 1) * G)
        ea = st.pop(f"ea{g}")
        # softmax denominator chain runs CONCURRENTLY with the (unnormalized)
        # AV matmuls; 1/Za folds into the PSUM-evac multiplies (or, when ACT
        # has slack, into a single Pool multiply with plain ACT copies).
        # za and zb share one PSUM bank (za's row is consumed by zarow
        # before the zb broadcast overwrites the bank).
        zt = self.lg_ps.tile([128, G], F32, tag="lg")
        nc.tensor.matmul(zt[:1, :], self.ones_colb, ea, start=True, stop=True)
        zarow = self.at_pool.tile([1, G], F32R, tag="zarow")
        nc.scalar.copy(out=zarow, in_=zt[:1, :])
        oc = self.oc_pool.tile([128, DC, G], F32, tag="oc")
        if evac_act:
            nc.tensor.matmul(zt, self.ones_row, zarow, start=True, stop=True)
            rzb = self.at_pool.tile([128, G], F32, tag="rzb")
            nc.vector.reciprocal(out=rzb, in_=zt)
            ean = self.at_pool.tile([Q, G], BF16, tag="ean")
            nc.gpsimd.tensor_mul(out=ean, in0=ea, in1=rzb)
            for jc in range(DC):
                ep = self.mm_ps.tile([128, G], F32, tag="mm")
                nc.tensor.matmul(ep, st["v_sb"][:, ts(jc, 128)], ean,
                                 start=True, stop=True)
                nc.scalar.copy(out=oc[:, jc, :], in_=ep)
        else:
            eps = []
            for jc in range(3):
                ep = self.mm_ps.tile([128, G], F32, tag="mm")
                nc.tensor.matmul(ep, st["v_sb"][:, ts(jc, 128)], ea,
                                 start=True, stop=True)
                eps.append(ep)
            nc.tensor.matmul(zt, self.ones_row, zarow, start=True, stop=True)
            rzb = self.at_pool.tile([128, G], F32, tag="rzb")
            nc.vector.reciprocal(out=rzb, in_=zt)
            for jc in range(3):
                nc.vector.tensor_mul(out=oc[:, jc, :], in0=eps[jc], in1=rzb)
            for jc in range(3, DC):
                ep = self.mm_ps.tile([128, G], F32, tag="mm")
                nc.tensor.matmul(ep, st["v_sb"][:, ts(jc, 128)], ea,
                                 start=True, stop=True)
                nc.vector.tensor_mul(out=oc[:, jc, :], in0=ep, in1=rzb)
        nc.sync.dma_start(
            out=self.out.ap()[b].rearrange("(c p) n -> p c n", p=128)[:, :, gs],
            in_=oc,
        )


_compiled = {}


def kernel(**inputs):
    visual_feat = np.ascontiguousarray(inputs["visual_feat"], dtype=np.float32)
    text_feat = np.ascontiguousarray(inputs["text_feat"], dtype=np.float32)
    Wq = np.ascontiguousarray(inputs["Wq"], dtype=np.float32)
    Wk = np.ascontiguousarray(inputs["Wk"], dtype=np.float32)
    Wv = np.ascontiguousarray(inputs["Wv"], dtype=np.float32)
    bq = np.ascontiguousarray(inputs["bq"], dtype=np.float32)
    bk = np.ascontiguousarray(inputs["bk"], dtype=np.float32)
    bv = np.ascontiguousarray(inputs["bv"], dtype=np.float32)

    f8 = ml_dtypes.float8_e4m3fn
    bf = ml_dtypes.bfloat16
    wq8 = (Wq * WSCALE).astype(f8)
    wkb = Wk.astype(bf)
    wvb = Wv.astype(bf)
    bq8 = (bq * WSCALE).astype(f8)
    bk_s = bk * WSCALE
    bvb = bv.astype(bf)

    vis = visual_feat.reshape(B, D, N)
    in_maps = []
    for c in range(N_CORES):
        bs = slice(c * BPC, (c + 1) * BPC)
        in_maps.append(
            {
                "visual": np.ascontiguousarray(vis[bs]),
                "text": np.ascontiguousarray(text_feat[bs]),
                "wq8": wq8, "wkb": wkb, "wvb": wvb,
                "bq8": bq8, "bk": bk_s, "bvb": bvb,
            }
        )

    if "nc" not in _compiled:
        nc = build_bass()
        nc.compile()
        _compiled["nc"] = nc
    res = run_bass_kernel_spmd(_compiled["nc"], in_maps, core_ids=list(range(N_CORES)))
    _compiled["last_result"] = res

    out = np.concatenate([r["out"] for r in res.results], axis=0)
    return out.reshape(B, D, HH, WW)


if __name__ == "__main__":
    nc = build_bass()
    nc.compile()
    print("build ok")
